# revision 17
# baseline (speedup 1.0000x reference)
"""Trainium2 Bass kernel for nn_CNNNer (sparse band biaffine NER scorer).

Math collapse (everything after the GELU stage is linear):
  head = gelu(state@Wh+bh) ++ [1]          (features i = 0..200, i=200 is the 1)
  tail = gelu(state@Wt+bt) ++ [1]
  band[n,r,k] = head[n]^T U''_k tail[m],  m = n+r-64
      with U''_k = U_k + e_200 Wtp[k,:] + Whp[k,:]^T e_200^T
  scores'[n,r,t] = head_masked[n]^T UW_t tail_masked[m],
      UW_t = sum_k Wd[k,t] U''_k            (precomputed on host, [9,201,201])
  scores = scores' + bd  (host), masked-out entries = bd exactly.

Device work per core (8 cores; core = (batch b, query quarter) of 256 queries):
  1. head|tail gelu MLPs computed transposed; psum accumulations
     interleaved per HID-chunk so the PE starts on the first chunk.
  2. step A: UhT_t[j, x] = sum_i UW[t,i,j] headT[i,x]        (9 tags)
  3. step B: S_t[x, m]  = sum_j UhT_t[j, x] tailT[j, m]      (full 128x256
     score windows per query-chunk; band diag extracted on host)

All IO is bf16 (tolerance 2e-2; measured ~5e-3). DMA facts (measured):
  - only sync (SP) and scalar (Activation) have hardware DGE queues;
    gpsimd is software DGE (~1us/start overhead, merges runs, ~150 B/ns)
  - descriptors spray across the 16 DMA engines only when the partition
    count is divisible by the spray factor -- a 73-row transfer pins to
    ONE engine (22.5 GB/s); pad uw's tail tile to 80 rows.
  - HWDGE load throughput is ~75-80 B/ns per queue regardless of
    descriptor size (>=512B); stores are much faster (~290 B/ns/queue).
  So x|w chunks 0-3 stream on the two HW queues (progressive MLP
  gating), chunks 4-5 ride one SWDGE transfer, and uw streams per
  tag-triple on the HW queues behind x/w, just ahead of step A's use.
"""

import os

import numpy as np

B, N, HID = 2, 1024, 768
BSZ = 200
W = 64
TAGS = 9
F = BSZ + 1  # 201 features incl the ones column
NQ = 256  # queries per core
NW = NQ + 2 * W  # 384 window positions per core
R = 2 * W + 1  # 129 band offsets
NCORES = 8
I2 = F - 128  # 73: second feature tile rows (i = 128..200)
I2P = 80  # padded partition count so the uw2 DMA sprays across engines
F2 = BSZ - 128  # 72: second MLP output tile rows
HT = NQ + NW  # 640: packed head|tail columns
XW = NW + 2 * BSZ  # 784: packed x|w chunk columns

_cache: dict = {}


def _build_nc():
    import concourse.bass as bass
    import concourse.mybir as mybir
    import concourse.tile as tile
    from concourse import bacc

    dt = mybir.dt
    f32 = dt.float32
    bf = dt.bfloat16

    nc = bacc.Bacc(
        "TRN2", target_bir_lowering=False, debug=False, enable_asserts=False
    )
    # per-chunk packed [x(384) | wh|wt(400)], partition-major
    xw = nc.dram_tensor("xw", [128, 6, XW], bf, kind="ExternalInput").ap()
    # UW pre-folded on host as [i, t, j], i padded 201->208 (128+80 split)
    uwd = nc.dram_tensor("uwd", [128 + I2P, TAGS, F], bf, kind="ExternalInput").ap()
    # mskb cols: 0:256 query-mask (window cols 64:320), 256:640 key-mask
    # (cols 0:384), 640:644 biases (bh1, bt1, bh2pad, bt2pad), pad to 648
    mskb = nc.dram_tensor("mskb", [128, 648], bf, kind="ExternalInput").ap()
    sout = nc.dram_tensor("sout", [2, 128, TAGS, NQ], bf, kind="ExternalOutput").ap()

    gelu = mybir.ActivationFunctionType.Gelu

    with tile.TileContext(nc) as tc:
        with (
            tc.tile_pool(name="sb", bufs=1) as sb,
            tc.tile_pool(name="ps_m", bufs=1, space="PSUM") as ps_m,
            tc.tile_pool(name="ps_a", bufs=3, space="PSUM") as ps_a,
            tc.tile_pool(name="ps_s", bufs=2, space="PSUM") as ps_s,
        ):
            xw_sb = sb.tile([128, 6, XW], bf)
            uw1 = sb.tile([128, TAGS, F], bf)
            uw2 = sb.tile([I2P, TAGS, F], bf)
            mk_sb = sb.tile([128, 648], bf)
            # packed [head(256) | tail(384)] per feature-half
            ht1 = sb.tile([128, HT], bf)
            ht2 = sb.tile([I2, HT], bf)
            # packed step-A output: [:, t, 0:256] = j 0:128, [0:73, t,
            # 256:512] = j 128:201 (mirrors the packed pa psum bank)
            uh = sb.tile([128, TAGS, 2 * NQ], bf)
            s_sb = sb.tile([128, 2, TAGS, NQ], bf)

            # ---- PE pstate warmup: junk matmuls reading not-yet-written
            # SBUF (no dependencies -> they run during the DMA preamble).
            # The DVFS ramp needs ~3us of continuous PE work to reach full
            # clock; without this the whole MLP runs at 1.2GHz. Outputs go
            # to ps_s banks whose first real use overwrites with start=True.
            for _ in range(7):
                pw = ps_s.tile([128, 2 * NQ], f32, tag="ps")
                nc.tensor.matmul(
                    pw[0:1, 0:NQ],
                    uh[:, 0, 0:1],
                    uh[:, 0, 0:NQ],
                    start=True,
                    stop=True,
                )

            # ---- loads: x/w chunks 0-3 in lockstep on the two HW queues
            # (the MLP consumes chunk pairs as they land), chunks 4-5 in one
            # SWDGE transfer ahead of the mask, uw per tag-triple behind ----
            wtile = sb.tile([1, 4], bf)
            for k in range(4):
                nc.sync.dma_start(out=xw_sb[:, k, 0:NW], in_=xw[:, k, 0:NW])
                nc.scalar.dma_start(out=xw_sb[:, k, NW:XW], in_=xw[:, k, NW:XW])
            nc.gpsimd.dma_start(out=xw_sb[:, 4:6, :], in_=xw[:, 4:6, :])
            nc.gpsimd.dma_start(out=mk_sb, in_=mskb)
            # ones-feature row (partition 72 is not engine-addressable);
            # mskb row 0 cols 0:640 is exactly the masked ones feature
            nc.scalar.dma_start(out=ht2[F2 : F2 + 1, :], in_=mskb[0:1, 0:HT])
            nc.scalar.dma_start(out=uw2[:, 0:3, :], in_=uwd[128 : 128 + I2P, 0:3, :])
            for k in range(3):
                nc.sync.dma_start(
                    out=uw1[:, 3 * k : 3 * k + 3, :], in_=uwd[0:128, 3 * k : 3 * k + 3, :]
                )
            # dummy activation: forces the gelu ACT table load onto the
            # scalar queue now, during the load phase, not before the first
            # real activation
            nc.scalar.activation(
                out=wtile[0:1, 0:1], in_=xw_sb[0:1, 0, 0:1], func=gelu
            )

            # ---- MLPs: o = gelu(W^T x + b); h1/t1/t2 interleaved per chunk
            # (3 psum banks) in chunk-arrival order; h2 reuses h1's bank
            # afterwards. Junk warmup matmuls (into ps_s banks, overwritten
            # later) keep the PE busy between chunk arrivals so the DVFS
            # pstate ramps to full clock during the load phase. ----
            pm_h = ps_m.tile([128, 2 * NQ], f32, tag="h")
            pm_t1 = ps_m.tile([128, NW], f32, tag="t1")
            pm_t2 = ps_m.tile([F2, NW], f32, tag="t2")
            order = (0, 1, 2, 4, 5, 3)
            nwarm = {0: 2, 1: 2, 2: 2, 4: 1}
            for i, k in enumerate(order):
                st, sp = (i == 0), (i == 5)
                xq = xw_sb[:, k, W : W + NQ]
                xf = xw_sb[:, k, 0:NW]
                wv = xw_sb[:, k, NW:XW]
                nc.tensor.matmul(pm_h[:, 0:NQ], wv[:, 0:128], xq, start=st, stop=sp)
                nc.tensor.matmul(pm_t1, wv[:, BSZ : BSZ + 128], xf, start=st, stop=sp)
                nc.tensor.matmul(pm_t2, wv[:, BSZ + 128 :], xf, start=st, stop=sp)
                for _ in range(nwarm.get(k, 0)):
                    pw = ps_s.tile([128, 2 * NQ], f32, tag="ps")
                    nc.tensor.matmul(
                        pw[0:1, 0:NQ],
                        xw_sb[:, 0, 0:1],
                        xw_sb[:, 0, 0:NQ],
                        start=True,
                        stop=True,
                    )
            for i, k in enumerate(order):
                nc.tensor.matmul(
                    pm_h[0:F2, NQ : 2 * NQ],
                    xw_sb[:, k, NW + 128 : NW + BSZ],
                    xw_sb[:, k, W : W + NQ],
                    start=(i == 0),
                    stop=(i == 5),
                )
            # acts+masks split head-first so step A can start before the
            # tail halves are finished
            nc.scalar.activation(
                out=ht1[:, 0:NQ], in_=pm_h[:, 0:NQ], func=gelu, bias=mk_sb[:, 640:641]
            )
            nc.scalar.activation(
                out=ht2[0:F2, 0:NQ],
                in_=pm_h[0:F2, NQ : 2 * NQ],
                func=gelu,
                bias=mk_sb[0:F2, 642:643],
            )
            nc.vector.tensor_mul(ht1[:, 0:NQ], ht1[:, 0:NQ], mk_sb[:, 0:NQ])
            nc.vector.tensor_mul(
                ht2[0:F2, 0:NQ], ht2[0:F2, 0:NQ], mk_sb[0:F2, 0:NQ]
            )
            nc.scalar.activation(
                out=ht1[:, NQ:HT], in_=pm_t1, func=gelu, bias=mk_sb[:, 641:642]
            )
            nc.scalar.activation(
                out=ht2[0:F2, NQ:HT], in_=pm_t2, func=gelu, bias=mk_sb[0:F2, 643:644]
            )
            nc.vector.tensor_mul(ht1[:, NQ:HT], ht1[:, NQ:HT], mk_sb[:, NQ:HT])
            nc.vector.tensor_mul(
                ht2[0:F2, NQ:HT], ht2[0:F2, NQ:HT], mk_sb[0:F2, NQ:HT]
            )
            # remaining uw2 triples trigger after the activations so the
            # scalar engine is free for the act path at the critical moment
            nc.scalar.dma_start(out=uw2[:, 3:6, :], in_=uwd[128 : 128 + I2P, 3:6, :])
            nc.scalar.dma_start(out=uw2[:, 6:9, :], in_=uwd[128 : 128 + I2P, 6:9, :])

            # ---- per tag: step A then step B, stores per tag-triple ----
            for t in range(TAGS):
                pa = ps_a.tile([128, 2 * NQ], f32, tag="pa")
                nc.tensor.matmul(
                    pa[:, 0:NQ], uw1[:, t, 0:128], ht1[:, 0:NQ], start=True, stop=False
                )
                nc.tensor.matmul(
                    pa[:, 0:NQ],
                    uw2[0:I2, t, 0:128],
                    ht2[:, 0:NQ],
                    start=False,
                    stop=True,
                )
                nc.tensor.matmul(
                    pa[0:I2, NQ : 2 * NQ],
                    uw1[:, t, 128:F],
                    ht1[:, 0:NQ],
                    start=True,
                    stop=False,
                )
                nc.tensor.matmul(
                    pa[0:I2, NQ : 2 * NQ],
                    uw2[0:I2, t, 128:F],
                    ht2[:, 0:NQ],
                    start=False,
                    stop=True,
                )
                nc.vector.tensor_copy(uh[:, t, :], pa)
                pS = ps_s.tile([128, 2 * NQ], f32, tag="ps")
                for qc in range(2):
                    q0 = qc * 128
                    nc.tensor.matmul(
                        pS[:, qc * NQ : qc * NQ + NQ],
                        uh[:, t, q0 : q0 + 128],
                        ht1[:, NQ + q0 : NQ + q0 + NQ],
                        start=True,
                        stop=False,
                    )
                    nc.tensor.matmul(
                        pS[:, qc * NQ : qc * NQ + NQ],
                        uh[0:I2, t, NQ + q0 : NQ + q0 + 128],
                        ht2[:, NQ + q0 : NQ + q0 + NQ],
                        start=False,
                        stop=True,
                    )
                nc.scalar.copy(out=s_sb[:, :, t, :], in_=pS)
                if t % 3 == 2:
                    # 1536B descriptors, 128 rows -> full engine spray;
                    # one HW queue per query-chunk
                    for qc in range(2):
                        (nc.sync, nc.scalar)[qc].dma_start(
                            out=sout[qc, :, t - 2 : t + 1, :],
                            in_=s_sb[:, qc, t - 2 : t + 1, :],
                        )

    nc.compile()
    return nc


def _get_nc():
    if "nc" not in _cache:
        _cache["nc"] = _build_nc()
    return _cache["nc"]


def _install_ntff_hook():
    """Profiling-only (BASSK_TRACE=1): provide antenv.axon_hooks if the
    image lacks it, wired to the libaxon NTFF capture via ctypes."""
    import sys
    import types

    try:
        from antenv.axon_hooks import get_axon_ntff_profile_hook  # noqa: F401

        return
    except ImportError:
        pass
    from trn_agent_boot.trn_boot import _ntff_profile_via_ctypes

    hook = _ntff_profile_via_ctypes("/opt/axon/libaxon_pjrt.so")
    mod = types.ModuleType("antenv.axon_hooks")
    mod._hook = hook
    mod.get_axon_ntff_profile_hook = lambda: mod._hook
    mod.set_axon_ntff_profile_hook = lambda h: setattr(mod, "_hook", h)
    sys.modules["antenv.axon_hooks"] = mod


def _host_prep(state, lengths, Wh, bh, Wt, bt, U, Wcat, Wd):
    """Fold U/Wcat/Wd into UW[9,201,201] and build per-core inputs."""
    import ml_dtypes

    bf = ml_dtypes.bfloat16
    Whp = Wcat[:, :F]  # [K, 201]
    Wtp = Wcat[:, F:]  # [K, 201]
    U2 = U.astype(np.float64).copy()
    U2[:, F - 1, :] += Wtp  # head ones-row picks up the tail term
    U2[:, :, F - 1] += Whp  # tail ones-col picks up the head term
    UW = np.einsum("kt,kij->tij", Wd.astype(np.float64), U2).astype(np.float32)
    uwp = np.zeros((128 + I2P, TAGS, F), np.float32)
    uwp[0:F] = UW.transpose(1, 0, 2)
    uwd = np.ascontiguousarray(uwp.astype(bf))
    # [HID, 400] -> partition-major [128, 6, 400]
    wcat = np.concatenate([Wh, Wt], axis=1).reshape(6, 128, 2 * BSZ)
    w2 = wcat.transpose(1, 0, 2)
    bias4 = np.stack(
        [
            bh[0:128],
            bt[0:128],
            np.pad(bh[128:BSZ], (0, 128 - F2)),
            np.pad(bt[128:BSZ], (0, 128 - F2)),
        ],
        axis=1,
    ).astype(np.float32)

    in_maps = []
    for b in range(B):
        for qi in range(N // NQ):
            q0 = qi * NQ
            lo = q0 - W
            x = np.zeros((NW, HID), np.float32)
            s, e = max(lo, 0), min(q0 + NQ + W, N)
            x[s - lo : e - lo] = state[b, s:e]
            x2 = x.T.reshape(6, 128, NW).transpose(1, 0, 2)
            xwp = np.empty((128, 6, XW), np.float32)
            xwp[:, :, 0:NW] = x2
            xwp[:, :, NW:XW] = w2
            pos = lo + np.arange(NW)
            mrow = ((pos >= 0) & (pos < N) & (pos < lengths[b])).astype(np.float32)
            mskb = np.zeros((128, 648), np.float32)
            mskb[:, 0:NQ] = mrow[None, W : W + NQ]
            mskb[:, NQ:HT] = mrow[None, :]
            mskb[:, 640:644] = bias4
            in_maps.append(
                {
                    "xw": np.ascontiguousarray(xwp.astype(bf)),
                    "uwd": uwd,
                    "mskb": np.ascontiguousarray(mskb.astype(bf)),
                }
            )
    return in_maps


def _assemble(outs, bd):
    """outs: NCORES arrays [2, 128, TAGS, NQ] -> scores [B, N, R, TAGS]."""
    scores = np.empty((B, N, R, TAGS), np.float32)
    mi = np.arange(128)[None, :, None, None] + np.arange(R)[None, None, None, :]
    for c, S in enumerate(outs):
        b, qi = divmod(c, N // NQ)
        g = np.take_along_axis(S.astype(np.float32), mi, axis=3)  # [2,128,TAGS,R]
        scores[b, qi * NQ : (qi + 1) * NQ] = g.reshape(NQ, TAGS, R).transpose(0, 2, 1)
    scores += bd.astype(np.float32)[None, None, None, :]
    return np.where(np.isfinite(scores), scores, 0.0).astype(np.float32)


def kernel(**inputs):
    state = np.asarray(inputs["state"], np.float32)
    lengths = np.asarray(inputs["lengths"]).astype(np.int64)
    Wh = np.ascontiguousarray(np.asarray(inputs["Wh"], np.float32))
    bh = np.asarray(inputs["bh"], np.float32)
    Wt = np.ascontiguousarray(np.asarray(inputs["Wt"], np.float32))
    bt = np.asarray(inputs["bt"], np.float32)
    U = np.asarray(inputs["U"], np.float32)
    Wcat = np.asarray(inputs["Wcat"], np.float32)
    Wd = np.asarray(inputs["Wd"], np.float32)
    bd = np.asarray(inputs["bd"], np.float32)

    in_maps = _host_prep(state, lengths, Wh, bh, Wt, bt, U, Wcat, Wd)
    nc = _get_nc()

    if os.environ.get("BASSK_SIM"):
        from concourse.bass_interp import CoreSim

        outs = []
        for im in in_maps:
            sim = CoreSim(nc, trace=False)
            for k, v in im.items():
                sim.tensor(k)[:] = v
            sim.simulate()
            outs.append(sim.tensor("sout").copy())
    else:
        trace = bool(os.environ.get("BASSK_TRACE"))
        if trace:
            _install_ntff_hook()
        from concourse.bass_utils import run_bass_kernel_spmd

        try:
            res = run_bass_kernel_spmd(
                nc, in_maps, core_ids=list(range(NCORES)), trace=trace
            )
        except Exception:
            # transient NRT/device hiccups recover on a fresh attempt
            import time

            time.sleep(2.0)
            res = run_bass_kernel_spmd(
                nc, in_maps, core_ids=list(range(NCORES)), trace=trace
            )
        _cache["last_result"] = res
        outs = [r["sout"] for r in res.results]

    return _assemble(outs, bd)


# revision 20
# speedup vs baseline: 1.0315x; 1.0315x over previous
"""Trainium2 Bass kernel for nn_CNNNer (sparse band biaffine NER scorer).

Math collapse (everything after the GELU stage is linear):
  head = gelu(state@Wh+bh) ++ [1]          (features i = 0..200, i=200 is the 1)
  tail = gelu(state@Wt+bt) ++ [1]
  band[n,r,k] = head[n]^T U''_k tail[m],  m = n+r-64
      with U''_k = U_k + e_200 Wtp[k,:] + Whp[k,:]^T e_200^T
  scores'[n,r,t] = head_masked[n]^T UW_t tail_masked[m],
      UW_t = sum_k Wd[k,t] U''_k            (precomputed on host, [9,201,201])
  scores = scores' + bd  (host), masked-out entries = bd exactly.

Device work per core (8 cores; core = (batch b, query quarter) of 256 queries):
  1. head|tail gelu MLPs computed transposed; psum accumulations
     interleaved per HID-chunk so the PE starts on the first chunk.
  2. step A: UhT_t[j, x] = sum_i UW[t,i,j] headT[i,x]        (9 tags)
  3. step B: S_t[x, m]  = sum_j UhT_t[j, x] tailT[j, m]      (full 128x256
     score windows per query-chunk; band diag extracted on host)

All IO is bf16 (tolerance 2e-2; measured ~5e-3). DMA facts (measured):
  - only sync (SP) and scalar (Activation) have hardware DGE queues;
    gpsimd is software DGE (~1us/start overhead, merges runs, ~150 B/ns)
  - descriptors spray across the 16 DMA engines only when the partition
    count is divisible by the spray factor -- a 73-row transfer pins to
    ONE engine (22.5 GB/s); pad uw's tail tile to 80 rows.
  - HWDGE load throughput is ~75-80 B/ns per queue regardless of
    descriptor size (>=512B); stores are much faster (~290 B/ns/queue).
  So x|w chunks 0-3 stream on the two HW queues (progressive MLP
  gating), chunks 4-5 ride one SWDGE transfer, and uw streams per
  tag-triple on the HW queues behind x/w, just ahead of step A's use.
"""

import os

import numpy as np

B, N, HID = 2, 1024, 768
BSZ = 200
W = 64
TAGS = 9
F = BSZ + 1  # 201 features incl the ones column
NQ = 256  # queries per core
NW = NQ + 2 * W  # 384 window positions per core
R = 2 * W + 1  # 129 band offsets
NCORES = 8
I2 = F - 128  # 73: second feature tile rows (i = 128..200)
I2P = 80  # padded partition count so the uw2 DMA sprays across engines
F2 = BSZ - 128  # 72: second MLP output tile rows
HT = NQ + NW  # 640: packed head|tail columns
XW = NW + 2 * BSZ  # 784: packed x|w chunk columns

_cache: dict = {}


def _build_nc():
    import concourse.bass as bass
    import concourse.mybir as mybir
    import concourse.tile as tile
    from concourse import bacc

    dt = mybir.dt
    f32 = dt.float32
    bf = dt.bfloat16

    nc = bacc.Bacc(
        "TRN2", target_bir_lowering=False, debug=False, enable_asserts=False
    )
    # per-chunk packed [x(384) | wh|wt(400)], partition-major
    xw = nc.dram_tensor("xw", [128, 6, XW], bf, kind="ExternalInput").ap()
    # UW pre-folded on host as [i, t, j], i padded 201->208 (128+80 split)
    uwd = nc.dram_tensor("uwd", [128 + I2P, TAGS, F], bf, kind="ExternalInput").ap()
    # mskb cols: 0:256 query-mask (window cols 64:320), 256:640 key-mask
    # (cols 0:384), 640:644 biases (bh1, bt1, bh2pad, bt2pad), pad to 648
    mskb = nc.dram_tensor("mskb", [128, 648], bf, kind="ExternalInput").ap()
    sout = nc.dram_tensor("sout", [2, 128, TAGS, NQ], bf, kind="ExternalOutput").ap()

    gelu = mybir.ActivationFunctionType.Gelu

    with tile.TileContext(nc) as tc:
        with (
            tc.tile_pool(name="sb", bufs=1) as sb,
            tc.tile_pool(name="ps_m", bufs=1, space="PSUM") as ps_m,
            tc.tile_pool(name="ps_a", bufs=3, space="PSUM") as ps_a,
            tc.tile_pool(name="ps_s", bufs=2, space="PSUM") as ps_s,
        ):
            xw_sb = sb.tile([128, 6, XW], bf)
            uw1 = sb.tile([128, TAGS, F], bf)
            uw2 = sb.tile([I2P, TAGS, F], bf)
            mk_sb = sb.tile([128, 648], bf)
            # packed [head(256) | tail(384)] per feature-half
            ht1 = sb.tile([128, HT], bf)
            ht2 = sb.tile([I2, HT], bf)
            # packed step-A output: [:, t, 0:256] = j 0:128, [0:73, t,
            # 256:512] = j 128:201 (mirrors the packed pa psum bank)
            uh = sb.tile([128, TAGS, 2 * NQ], bf)
            s_sb = sb.tile([128, 2, TAGS, NQ], bf)

            # ---- loads: x/w chunks 0-3 in lockstep on the two HW queues
            # (the MLP consumes chunk pairs as they land), chunks 4-5 in one
            # SWDGE transfer ahead of the mask, uw per tag-triple behind ----
            wtile = sb.tile([1, 4], bf)
            for k in range(4):
                nc.sync.dma_start(out=xw_sb[:, k, 0:NW], in_=xw[:, k, 0:NW])
                nc.scalar.dma_start(out=xw_sb[:, k, NW:XW], in_=xw[:, k, NW:XW])
            nc.gpsimd.dma_start(out=xw_sb[:, 4:6, :], in_=xw[:, 4:6, :])
            nc.gpsimd.dma_start(out=mk_sb, in_=mskb)
            # ones-feature row (partition 72 is not engine-addressable);
            # mskb row 0 cols 0:640 is exactly the masked ones feature
            nc.scalar.dma_start(out=ht2[F2 : F2 + 1, :], in_=mskb[0:1, 0:HT])
            nc.scalar.dma_start(out=uw2[:, 0:3, :], in_=uwd[128 : 128 + I2P, 0:3, :])
            for k in range(3):
                nc.sync.dma_start(
                    out=uw1[:, 3 * k : 3 * k + 3, :], in_=uwd[0:128, 3 * k : 3 * k + 3, :]
                )
            # dummy activation: forces the gelu ACT table load onto the
            # scalar queue now, during the load phase, not before the first
            # real activation
            nc.scalar.activation(
                out=wtile[0:1, 0:1], in_=xw_sb[0:1, 0, 0:1], func=gelu
            )

            # ---- MLPs: o = gelu(W^T x + b); h1/t1/t2 interleaved per chunk
            # (3 psum banks) in chunk-arrival order; h2 reuses h1's bank
            # afterwards. Junk warmup matmuls (into ps_s banks, overwritten
            # later) keep the PE busy between chunk arrivals so the DVFS
            # pstate ramps to full clock during the load phase. ----
            pm_h = ps_m.tile([128, 2 * NQ], f32, tag="h")
            pm_t1 = ps_m.tile([128, NW], f32, tag="t1")
            pm_t2 = ps_m.tile([F2, NW], f32, tag="t2")
            order = (0, 1, 2, 4, 5, 3)
            nwarm = {0: 2, 1: 2, 2: 2, 4: 1}
            for i, k in enumerate(order):
                st, sp = (i == 0), (i == 5)
                xq = xw_sb[:, k, W : W + NQ]
                xf = xw_sb[:, k, 0:NW]
                wv = xw_sb[:, k, NW:XW]
                nc.tensor.matmul(pm_h[:, 0:NQ], wv[:, 0:128], xq, start=st, stop=sp)
                nc.tensor.matmul(pm_t1, wv[:, BSZ : BSZ + 128], xf, start=st, stop=sp)
                nc.tensor.matmul(pm_t2, wv[:, BSZ + 128 :], xf, start=st, stop=sp)
                for _ in range(nwarm.get(k, 0)):
                    pw = ps_s.tile([128, 2 * NQ], f32, tag="ps")
                    nc.tensor.matmul(
                        pw[0:1, 0:NQ],
                        xw_sb[:, 0, 0:1],
                        xw_sb[:, 0, 0:NQ],
                        start=True,
                        stop=True,
                    )
            for i, k in enumerate(order):
                nc.tensor.matmul(
                    pm_h[0:F2, NQ : 2 * NQ],
                    xw_sb[:, k, NW + 128 : NW + BSZ],
                    xw_sb[:, k, W : W + NQ],
                    start=(i == 0),
                    stop=(i == 5),
                )
            # acts+masks split head-first so step A can start before the
            # tail halves are finished
            nc.scalar.activation(
                out=ht1[:, 0:NQ], in_=pm_h[:, 0:NQ], func=gelu, bias=mk_sb[:, 640:641]
            )
            nc.scalar.activation(
                out=ht2[0:F2, 0:NQ],
                in_=pm_h[0:F2, NQ : 2 * NQ],
                func=gelu,
                bias=mk_sb[0:F2, 642:643],
            )
            nc.vector.tensor_mul(ht1[:, 0:NQ], ht1[:, 0:NQ], mk_sb[:, 0:NQ])
            nc.vector.tensor_mul(
                ht2[0:F2, 0:NQ], ht2[0:F2, 0:NQ], mk_sb[0:F2, 0:NQ]
            )
            nc.scalar.activation(
                out=ht1[:, NQ:HT], in_=pm_t1, func=gelu, bias=mk_sb[:, 641:642]
            )
            nc.scalar.activation(
                out=ht2[0:F2, NQ:HT], in_=pm_t2, func=gelu, bias=mk_sb[0:F2, 643:644]
            )
            nc.vector.tensor_mul(ht1[:, NQ:HT], ht1[:, NQ:HT], mk_sb[:, NQ:HT])
            nc.vector.tensor_mul(
                ht2[0:F2, NQ:HT], ht2[0:F2, NQ:HT], mk_sb[0:F2, NQ:HT]
            )
            # remaining uw2 triples trigger after the activations so the
            # scalar engine is free for the act path at the critical moment
            nc.scalar.dma_start(out=uw2[:, 3:6, :], in_=uwd[128 : 128 + I2P, 3:6, :])
            nc.scalar.dma_start(out=uw2[:, 6:9, :], in_=uwd[128 : 128 + I2P, 6:9, :])

            # ---- per tag: step A then step B, stores per tag-triple ----
            for t in range(TAGS):
                pa = ps_a.tile([128, 2 * NQ], f32, tag="pa")
                nc.tensor.matmul(
                    pa[:, 0:NQ], uw1[:, t, 0:128], ht1[:, 0:NQ], start=True, stop=False
                )
                nc.tensor.matmul(
                    pa[:, 0:NQ],
                    uw2[0:I2, t, 0:128],
                    ht2[:, 0:NQ],
                    start=False,
                    stop=True,
                )
                nc.tensor.matmul(
                    pa[0:I2, NQ : 2 * NQ],
                    uw1[:, t, 128:F],
                    ht1[:, 0:NQ],
                    start=True,
                    stop=False,
                )
                nc.tensor.matmul(
                    pa[0:I2, NQ : 2 * NQ],
                    uw2[0:I2, t, 128:F],
                    ht2[:, 0:NQ],
                    start=False,
                    stop=True,
                )
                nc.vector.tensor_copy(uh[:, t, :], pa)
                pS = ps_s.tile([128, 2 * NQ], f32, tag="ps")
                for qc in range(2):
                    q0 = qc * 128
                    nc.tensor.matmul(
                        pS[:, qc * NQ : qc * NQ + NQ],
                        uh[:, t, q0 : q0 + 128],
                        ht1[:, NQ + q0 : NQ + q0 + NQ],
                        start=True,
                        stop=False,
                    )
                    nc.tensor.matmul(
                        pS[:, qc * NQ : qc * NQ + NQ],
                        uh[0:I2, t, NQ + q0 : NQ + q0 + 128],
                        ht2[:, NQ + q0 : NQ + q0 + NQ],
                        start=False,
                        stop=True,
                    )
                if t < TAGS - 1:
                    nc.scalar.copy(out=s_sb[:, :, t, :], in_=pS)
                else:
                    # last tag: copy+store per query-chunk so the qc0 store
                    # overlaps the qc1 epilogue (shortens the final chain)
                    for qc in range(2):
                        nc.scalar.copy(
                            out=s_sb[:, qc, t, :], in_=pS[:, qc * NQ : qc * NQ + NQ]
                        )
                        (nc.sync, nc.scalar)[qc].dma_start(
                            out=sout[qc, :, t : t + 1, :],
                            in_=s_sb[:, qc, t : t + 1, :],
                        )
                if t in (2, 5, 7):
                    # 1024-1536B descriptors, 128 rows -> full engine spray;
                    # one HW queue per query-chunk
                    lo = t - 2 if t != 7 else 6
                    for qc in range(2):
                        (nc.sync, nc.scalar)[qc].dma_start(
                            out=sout[qc, :, lo : t + 1, :],
                            in_=s_sb[:, qc, lo : t + 1, :],
                        )

    nc.compile()
    return nc


def _get_nc():
    if "nc" not in _cache:
        _cache["nc"] = _build_nc()
    return _cache["nc"]


def _install_ntff_hook():
    """Profiling-only (BASSK_TRACE=1): provide antenv.axon_hooks if the
    image lacks it, wired to the libaxon NTFF capture via ctypes."""
    import sys
    import types

    try:
        from antenv.axon_hooks import get_axon_ntff_profile_hook  # noqa: F401

        return
    except ImportError:
        pass
    from trn_agent_boot.trn_boot import _ntff_profile_via_ctypes

    hook = _ntff_profile_via_ctypes("/opt/axon/libaxon_pjrt.so")
    mod = types.ModuleType("antenv.axon_hooks")
    mod._hook = hook
    mod.get_axon_ntff_profile_hook = lambda: mod._hook
    mod.set_axon_ntff_profile_hook = lambda h: setattr(mod, "_hook", h)
    sys.modules["antenv.axon_hooks"] = mod


def _host_prep(state, lengths, Wh, bh, Wt, bt, U, Wcat, Wd):
    """Fold U/Wcat/Wd into UW[9,201,201] and build per-core inputs."""
    import ml_dtypes

    bf = ml_dtypes.bfloat16
    Whp = Wcat[:, :F]  # [K, 201]
    Wtp = Wcat[:, F:]  # [K, 201]
    U2 = U.astype(np.float64).copy()
    U2[:, F - 1, :] += Wtp  # head ones-row picks up the tail term
    U2[:, :, F - 1] += Whp  # tail ones-col picks up the head term
    UW = np.einsum("kt,kij->tij", Wd.astype(np.float64), U2).astype(np.float32)
    uwp = np.zeros((128 + I2P, TAGS, F), np.float32)
    uwp[0:F] = UW.transpose(1, 0, 2)
    uwd = np.ascontiguousarray(uwp.astype(bf))
    # [HID, 400] -> partition-major [128, 6, 400]
    wcat = np.concatenate([Wh, Wt], axis=1).reshape(6, 128, 2 * BSZ)
    w2 = wcat.transpose(1, 0, 2)
    bias4 = np.stack(
        [
            bh[0:128],
            bt[0:128],
            np.pad(bh[128:BSZ], (0, 128 - F2)),
            np.pad(bt[128:BSZ], (0, 128 - F2)),
        ],
        axis=1,
    ).astype(np.float32)

    in_maps = []
    for b in range(B):
        for qi in range(N // NQ):
            q0 = qi * NQ
            lo = q0 - W
            x = np.zeros((NW, HID), np.float32)
            s, e = max(lo, 0), min(q0 + NQ + W, N)
            x[s - lo : e - lo] = state[b, s:e]
            x2 = x.T.reshape(6, 128, NW).transpose(1, 0, 2)
            xwp = np.empty((128, 6, XW), np.float32)
            xwp[:, :, 0:NW] = x2
            xwp[:, :, NW:XW] = w2
            pos = lo + np.arange(NW)
            mrow = ((pos >= 0) & (pos < N) & (pos < lengths[b])).astype(np.float32)
            mskb = np.zeros((128, 648), np.float32)
            mskb[:, 0:NQ] = mrow[None, W : W + NQ]
            mskb[:, NQ:HT] = mrow[None, :]
            mskb[:, 640:644] = bias4
            in_maps.append(
                {
                    "xw": np.ascontiguousarray(xwp.astype(bf)),
                    "uwd": uwd,
                    "mskb": np.ascontiguousarray(mskb.astype(bf)),
                }
            )
    return in_maps


def _assemble(outs, bd):
    """outs: NCORES arrays [2, 128, TAGS, NQ] -> scores [B, N, R, TAGS]."""
    scores = np.empty((B, N, R, TAGS), np.float32)
    mi = np.arange(128)[None, :, None, None] + np.arange(R)[None, None, None, :]
    for c, S in enumerate(outs):
        b, qi = divmod(c, N // NQ)
        g = np.take_along_axis(S.astype(np.float32), mi, axis=3)  # [2,128,TAGS,R]
        scores[b, qi * NQ : (qi + 1) * NQ] = g.reshape(NQ, TAGS, R).transpose(0, 2, 1)
    scores += bd.astype(np.float32)[None, None, None, :]
    return np.where(np.isfinite(scores), scores, 0.0).astype(np.float32)


def kernel(**inputs):
    state = np.asarray(inputs["state"], np.float32)
    lengths = np.asarray(inputs["lengths"]).astype(np.int64)
    Wh = np.ascontiguousarray(np.asarray(inputs["Wh"], np.float32))
    bh = np.asarray(inputs["bh"], np.float32)
    Wt = np.ascontiguousarray(np.asarray(inputs["Wt"], np.float32))
    bt = np.asarray(inputs["bt"], np.float32)
    U = np.asarray(inputs["U"], np.float32)
    Wcat = np.asarray(inputs["Wcat"], np.float32)
    Wd = np.asarray(inputs["Wd"], np.float32)
    bd = np.asarray(inputs["bd"], np.float32)

    in_maps = _host_prep(state, lengths, Wh, bh, Wt, bt, U, Wcat, Wd)
    nc = _get_nc()

    if os.environ.get("BASSK_SIM"):
        from concourse.bass_interp import CoreSim

        outs = []
        for im in in_maps:
            sim = CoreSim(nc, trace=False)
            for k, v in im.items():
                sim.tensor(k)[:] = v
            sim.simulate()
            outs.append(sim.tensor("sout").copy())
    else:
        trace = bool(os.environ.get("BASSK_TRACE"))
        if trace:
            _install_ntff_hook()
        from concourse.bass_utils import run_bass_kernel_spmd

        try:
            res = run_bass_kernel_spmd(
                nc, in_maps, core_ids=list(range(NCORES)), trace=trace
            )
        except Exception:
            # transient NRT/device hiccups recover on a fresh attempt
            import time

            time.sleep(2.0)
            res = run_bass_kernel_spmd(
                nc, in_maps, core_ids=list(range(NCORES)), trace=trace
            )
        _cache["last_result"] = res
        outs = [r["sout"] for r in res.results]

    return _assemble(outs, bd)
